# revision 4
# baseline (speedup 1.0000x reference)
"""BitNet MLP (nn_BitNetMLP) Trainium2 kernel — 8-core tensor-parallel over
the intermediate dimension I.

Math (reference):
  xq    = int4_absmean_quant(x)          per-token over H
  gate  = xq @ (ternary(w_gate)*wm_g).T
  up    = xq @ (ternary(w_up)*wm_u).T
  inter = int8_absmax_quant(up * relu(gate)^2)   per-token over I
  out   = inter @ (ternary(w_down)*wm_d).T

All quantized values are small integers; mm1 runs fp8 DoubleRow (xq in
[-8,7], ternary weights), mm2 runs bf16 (int8 inter exact in bf16).
Scales fold into per-token scalars applied on PSUM evacuation.
Rounding = fp32 magic-number trick (RNE); clip post-round in magic space.

Sharding: each core holds I/8 rows of w_gate/w_up, I/8 cols of w_down, full
x.  Collectives: per-weight AllReduce(add) of |w| sums, per-super-block
AllReduce(max) of per-token gamma partials, per-super ReduceScatter(add) of
bf16 [TSUP, H] output partials.

v2 structure (vs baseline): p (mm1 output) stays in SBUF; xqT kept as two
half-super fp8 SBUF buffers fed by DMA-transpose; w_down ternary transposed
ONCE into DRAM (wdT_d) and streamed as contiguous [128,512] slabs; mm2
processes super-PAIRS (1024 tokens per wd sweep) to halve wd re-reads;
partials and ReduceScatter in bf16.
"""

import numpy as np


def _seed_axon_hooks():
    import sys, types
    try:
        import antenv.axon_hooks  # noqa: F401
    except Exception:
        try:
            import antenv
        except Exception:
            return
        m = types.ModuleType("antenv.axon_hooks")
        m.get_axon_ntff_profile_hook = lambda: None
        m.set_axon_ntff_profile_hook = lambda h: None
        sys.modules["antenv.axon_hooks"] = m
        antenv.axon_hooks = m


_seed_axon_hooks()

N_CORES = 8
B, S = 2, 2048
H, I = 4096, 11008
T = B * S                    # 4096 tokens
IC = I // N_CORES            # 1376 intermediate per core
NSLAB = 11                   # ceil(1376/128) i-slabs for mm2
ICP = NSLAB * 128            # 1408 padded
TSUP = 512                   # tokens per super-block
NSUP = T // TSUP             # 8
TT = 128                     # tokens per tile
NTS = TSUP // TT             # 4 t_tiles per super
NT = T // TT                 # 32 global t_tiles
HB = H // 128                # 32 h-blocks
KP = HB // 2                 # 16 k-pairs (DoubleRow contraction pairs)
HH = H // 2                  # 2048
THIRDS = [(0, 512), (512, 512), (1024, IC - 1024)]   # i-splits for mm1 psum
MAGIC = 12582912.0           # 1.5 * 2^23 fp32 RNE trick
EPS = 1e-5
SQRT7 = float(np.sqrt(7.0))

_cache = {}


def _build(debug=False):
    import contextlib
    import concourse.mybir as mybir
    import concourse.tile as tile
    from concourse import bacc, bass_isa

    dt = mybir.dt
    Alu = mybir.AluOpType
    Act = mybir.ActivationFunctionType

    nc = bacc.Bacc("TRN2", target_bir_lowering=False, debug=False,
                   num_devices=N_CORES)

    x_in = nc.dram_tensor("x", [T, H], dt.float32, kind="ExternalInput")
    wg_in = nc.dram_tensor("wg", [IC, H], dt.float32, kind="ExternalInput")
    wu_in = nc.dram_tensor("wu", [IC, H], dt.float32, kind="ExternalInput")
    wd_in = nc.dram_tensor("wd", [H, IC], dt.float32, kind="ExternalInput")
    sc_in = nc.dram_tensor("scales", [1, 3], dt.float32, kind="ExternalInput")
    out_ext = nc.dram_tensor("out_rs", [NSUP, TSUP // N_CORES, H], dt.bfloat16,
                             kind="ExternalOutput")
    if debug:
        dbg = {
            "dbg_beta": nc.dram_tensor("dbg_beta", [128, NT], dt.float32, kind="ExternalOutput"),
            "dbg_gam": nc.dram_tensor("dbg_gam", [128, NT], dt.float32, kind="ExternalOutput"),
            "dbg_s2": nc.dram_tensor("dbg_s2", [128, NT], dt.float32, kind="ExternalOutput"),
            "dbg_Dt": nc.dram_tensor("dbg_Dt", [128, NT], dt.float32, kind="ExternalOutput"),
            "dbg_wm": nc.dram_tensor("dbg_wm", [1, 3], dt.float32, kind="ExternalOutput"),
            "dbg_xq": nc.dram_tensor("dbg_xq", [TSUP, H], dt.bfloat16, kind="ExternalOutput"),
            "dbg_iq": nc.dram_tensor("dbg_iq", [TSUP, ICP], dt.bfloat16, kind="ExternalOutput"),
            "dbg_part": nc.dram_tensor("dbg_part", [TSUP, H], dt.bfloat16, kind="ExternalOutput"),
        }

    RG = [list(range(N_CORES))]

    def row_tiles(rows, step=128):
        out, r0 = [], 0
        while r0 < rows:
            out.append((r0, min(step, rows - r0)))
            r0 += step
        return out

    with tile.TileContext(nc) as tc:
        ctx = contextlib.ExitStack()
        with ctx:
            dram = ctx.enter_context(tc.tile_pool(name="dram", bufs=1, space="DRAM"))
            wgt_d = dram.tile([IC, H], dt.bfloat16, tag="wgt")
            wut_d = dram.tile([IC, H], dt.bfloat16, tag="wut")
            wdt_d = dram.tile([H, IC], dt.bfloat16, tag="wdt")
            wdT_d = dram.tile([ICP, H], dt.bfloat16, tag="wdT")
            xq_d = [dram.tile([TSUP, H], dt.bfloat16, tag=f"xq{b}", name=f"xq_d{b}")
                    for b in range(NSUP)]
            iq_d = [dram.tile([TSUP, ICP], dt.bfloat16, tag=f"iq{b}", name=f"iq_d{b}")
                    for b in range(NSUP)]
            part_d = [dram.tile([TSUP, H], dt.bfloat16, tag=f"part{b}", name=f"part_d{b}")
                      for b in range(NSUP)]
            rs_d = [dram.tile([TSUP // N_CORES, H], dt.bfloat16, tag=f"rs{b}",
                              name=f"rs_d{b}") for b in range(NSUP)]
            ws_d = [dram.tile([1, 1], dt.float32, tag=f"ws{w}", name=f"ws_d{w}")
                    for w in range(3)]
            ws_a = [dram.tile([1, 1], dt.float32, tag=f"wsa{w}", name=f"ws_a{w}")
                    for w in range(3)]
            gpart_d = [dram.tile([128, NTS], dt.float32, tag=f"gpart{b}",
                                 name=f"gpart_d{b}") for b in range(NSUP)]
            gall_d = [dram.tile([128, NTS], dt.float32, tag=f"gall{b}",
                                name=f"gall_d{b}") for b in range(NSUP)]

            # ---------- SBUF pools ----------
            pwq8 = ctx.enter_context(tc.tile_pool(name="wq8", bufs=1))
            pxqt = ctx.enter_context(tc.tile_pool(name="xqt", bufs=2))
            ppx = ctx.enter_context(tc.tile_pool(name="px", bufs=2))
            pbst = ctx.enter_context(tc.tile_pool(name="bst", bufs=2))
            ptst = ctx.enter_context(tc.tile_pool(name="tst", bufs=3))
            psm = ctx.enter_context(tc.tile_pool(name="sm", bufs=1))
            pr = ctx.enter_context(tc.tile_pool(name="pr", bufs=2))
            pp_pool = ctx.enter_context(tc.tile_pool(name="pp", bufs=6))
            piqT = ctx.enter_context(tc.tile_pool(name="piqT", bufs=2))
            ppiq = ctx.enter_context(tc.tile_pool(name="ppiq", bufs=2))
            pwd = ctx.enter_context(tc.tile_pool(name="pwdsl", bufs=3))
            pev = ctx.enter_context(tc.tile_pool(name="pev", bufs=2))
            pwt = ctx.enter_context(tc.tile_pool(name="pwt", bufs=2))
            pps = ctx.enter_context(tc.tile_pool(name="ps", bufs=8, space="PSUM"))

            # gate/up fp8 transposed: col ((wi*HB+hb)*IC + i)
            wq8 = pwq8.tile([128, 2 * HB * IC], dt.float8e4, tag="wbig")

            scs = psm.tile([1, 3], dt.float32, tag="scs")
            nc.sync.dma_start(scs[:], sc_in.ap())
            sbc = psm.tile([128, 3], dt.float32, tag="sbc")
            nc.gpsimd.partition_broadcast(sbc[:], scs[:])
            wacc = psm.tile([128, 3], dt.float32, tag="wacc")
            nc.vector.memset(wacc[:], 0.0)
            beta_all = psm.tile([128, NT], dt.float32, tag="beta_all")
            gam_p = psm.tile([128, NT], dt.float32, tag="gam_p")
            gam = psm.tile([128, NT], dt.float32, tag="gam")
            s2 = psm.tile([128, NT], dt.float32, tag="s2")
            Dt = psm.tile([128, NT], dt.float32, tag="Dt")
            wred = psm.tile([128, 3], dt.float32, tag="wred")
            wsb = psm.tile([1, 3], dt.float32, tag="wsb")
            wsbc = psm.tile([128, 3], dt.float32, tag="wsbc")
            wmv = psm.tile([128, 3], dt.float32, tag="wmv")
            wrec = psm.tile([128, 3], dt.float32, tag="wrec")
            cgg = psm.tile([128, 1], dt.float32, tag="cgg")
            cuu = psm.tile([128, 1], dt.float32, tag="cuu")
            cdd = psm.tile([128, 1], dt.float32, tag="cdd")
            cb = psm.tile([128, 1], dt.float32, tag="cb")

            pts = {}

            # ================= x int4 quant =================================
            def prep_x_quant(b, ti):
                t = b * NTS + ti
                t0 = t * TT
                xh = [ppx.tile([128, HH], dt.float32, tag="px",
                               name=f"xh{t}_{h}") for h in range(2)]
                ac = [pr.tile([128, 1], dt.float32, tag="acc",
                              name=f"ac{t}_{h}") for h in range(2)]
                for h in range(2):
                    nc.sync.dma_start(xh[h][:], x_in.ap()[t0:t0 + TT,
                                                          h * HH:(h + 1) * HH])
                    nc.vector.tensor_reduce(out=ac[h][:], in_=xh[h][:],
                                            axis=mybir.AxisListType.X, op=Alu.add,
                                            apply_absolute_value=True)
                asum = pr.tile([128, 1], dt.float32, tag="asum", name=f"as{t}")
                nc.vector.tensor_tensor(out=asum[:], in0=ac[0][:], in1=ac[1][:],
                                        op=Alu.add)
                nc.vector.tensor_scalar(out=beta_all[:, t:t + 1], in0=asum[:],
                                        scalar1=1.0 / H, scalar2=None, op0=Alu.mult)
                dbe = pr.tile([128, 1], dt.float32, tag="dbe", name=f"db{t}")
                nc.vector.tensor_scalar(out=dbe[:], in0=asum[:], scalar1=1.0 / H,
                                        scalar2=EPS, op0=Alu.mult, op1=Alu.add)
                rbe = pr.tile([128, 1], dt.float32, tag="rbe", name=f"rb{t}")
                nc.vector.reciprocal(rbe[:], dbe[:])
                sbe = pr.tile([128, 1], dt.float32, tag="sbe", name=f"sb{t}")
                nc.vector.tensor_scalar(out=sbe[:], in0=rbe[:], scalar1=SQRT7,
                                        scalar2=None, op0=Alu.mult)
                for h in range(2):
                    nc.scalar.activation(xh[h][:], xh[h][:], Act.Copy, bias=MAGIC,
                                         scale=sbe[:])
                    nc.vector.tensor_scalar(out=xh[h][:], in0=xh[h][:],
                                            scalar1=MAGIC + 7.0, scalar2=MAGIC - 8.0,
                                            op0=Alu.min, op1=Alu.max)
                    xqh = pbst.tile([128, HH], dt.bfloat16, tag="bst",
                                    name=f"xq{t}_{h}")
                    nc.vector.tensor_scalar(out=xqh[:], in0=xh[h][:],
                                            scalar1=-MAGIC, scalar2=None, op0=Alu.add)
                    nc.sync.dma_start(xq_d[b][ti * TT:(ti + 1) * TT,
                                              h * HH:(h + 1) * HH], xqh[:])

            def prep_x_transpose(b, th):
                xt = pxqt.tile([128, HB * 2 * TT], dt.float8e4, tag="xqt",
                               name=f"xqT{b}_{th}")
                r0 = th * 2 * TT
                for hb in range(HB):
                    stg = ptst.tile([128, 2 * TT], dt.bfloat16, tag="tst",
                                    name=f"xstg{b}_{th}_{hb}")
                    nc.scalar.dma_start_transpose(
                        stg[:], xq_d[b][r0:r0 + 2 * TT, hb * 128:(hb + 1) * 128])
                    nc.vector.tensor_copy(xt[:, hb * 2 * TT:(hb + 1) * 2 * TT], stg[:])
                return xt

            def prep_x(b):
                for ti in range(NTS):
                    prep_x_quant(b, ti)
                return [prep_x_transpose(b, 0), prep_x_transpose(b, 1)]

            # ================= weight prep ==================================
            def w_scan(win, rows, wi):
                cols = win.shape[1]
                for r0, rr in row_tiles(rows):
                    for c0 in range(0, cols, HH):
                        cc = min(HH, cols - c0)
                        wt = ppx.tile([128, HH], dt.float32, tag="px",
                                      name=f"wt{wi}_{r0}_{c0}")
                        nc.sync.dma_start(wt[:rr, :cc], win.ap()[r0:r0 + rr, c0:c0 + cc])
                        acc = pr.tile([128, 1], dt.float32, tag="acc",
                                      name=f"wacc{wi}_{r0}_{c0}")
                        nc.vector.tensor_reduce(out=acc[:rr, :], in_=wt[:rr, :cc],
                                                axis=mybir.AxisListType.X, op=Alu.add,
                                                apply_absolute_value=True)
                        nc.vector.tensor_tensor(
                            out=wacc[:rr, wi:wi + 1], in0=wacc[:rr, wi:wi + 1],
                            in1=acc[:rr, :], op=Alu.add)
                nc.gpsimd.partition_all_reduce(wred[:, wi:wi + 1], wacc[:, wi:wi + 1],
                                               channels=128,
                                               reduce_op=bass_isa.ReduceOp.add)
                nc.sync.dma_start(ws_d[wi][:], wred[0:1, wi:wi + 1])
                nc.gpsimd.collective_compute("AllReduce", Alu.add, replica_groups=RG,
                                             ins=[ws_d[wi].opt()], outs=[ws_a[wi].opt()])
                nc.gpsimd.dma_start(wsb[:, wi:wi + 1], ws_a[wi][:])
                nc.gpsimd.partition_broadcast(wsbc[:, wi:wi + 1], wsb[:, wi:wi + 1])
                nc.vector.tensor_scalar(out=wmv[:, wi:wi + 1], in0=wsbc[:, wi:wi + 1],
                                        scalar1=1.0 / (I * H), scalar2=None,
                                        op0=Alu.mult)
                nc.vector.tensor_scalar(out=wred[:, wi:wi + 1], in0=wsbc[:, wi:wi + 1],
                                        scalar1=1.0 / (I * H), scalar2=EPS,
                                        op0=Alu.mult, op1=Alu.add)
                nc.vector.reciprocal(wrec[:, wi:wi + 1], wred[:, wi:wi + 1])

            def w_quant(win, rows, wi, wdst):
                cols = win.shape[1]
                for r0, rr in row_tiles(rows):
                    for c0 in range(0, cols, HH):
                        cc = min(HH, cols - c0)
                        wt = ppx.tile([128, HH], dt.float32, tag="px",
                                      name=f"wq{wi}_{r0}_{c0}")
                        nc.sync.dma_start(wt[:rr, :cc], win.ap()[r0:r0 + rr, c0:c0 + cc])
                        nc.scalar.activation(wt[:rr, :cc], wt[:rr, :cc], Act.Copy,
                                             bias=MAGIC, scale=wrec[:rr, wi:wi + 1])
                        nc.vector.tensor_scalar(out=wt[:rr, :cc], in0=wt[:rr, :cc],
                                                scalar1=MAGIC + 1.0, scalar2=MAGIC - 1.0,
                                                op0=Alu.min, op1=Alu.max)
                        wq = pbst.tile([128, HH], dt.bfloat16, tag="bst",
                                       name=f"wqo{wi}_{r0}_{c0}")
                        nc.vector.tensor_scalar(out=wq[:rr, :cc], in0=wt[:rr, :cc],
                                                scalar1=-MAGIC, scalar2=None,
                                                op0=Alu.add)
                        nc.sync.dma_start(wdst[r0:r0 + rr, c0:c0 + cc], wq[:rr, :cc])

            def w_transpose_gu(wi, wsrc):
                for hb in range(HB):
                    stg = pwt.tile([128, HH], dt.bfloat16, tag="wt",
                                   name=f"wstg{wi}_{hb}")
                    nc.scalar.dma_start_transpose(stg[:, :IC],
                                                  wsrc[:, hb * 128:(hb + 1) * 128])
                    off = (wi * HB + hb) * IC
                    nc.vector.tensor_copy(wq8[:, off:off + IC], stg[:, :IC])

            def w_transpose_d():
                for ib in range(NSLAB):
                    i0 = ib * 128
                    iw = min(128, IC - i0)
                    for hh in range(2):
                        stg = pwt.tile([128, HH], dt.bfloat16, tag="wt",
                                       name=f"wdstg{ib}_{hh}")
                        nc.scalar.dma_start_transpose(
                            stg[:iw, :], wdt_d[hh * HH:(hh + 1) * HH, i0:i0 + iw])
                        if iw < 128:
                            nc.vector.memset(stg[iw:128, :], 0.0)
                        nc.sync.dma_start(wdT_d[i0:i0 + 128, hh * HH:(hh + 1) * HH],
                                          stg[:])

            # ===================== mm1 ======================================
            def mm1_super(b, xts):
                for ti in range(NTS):
                    t = b * NTS + ti
                    xt = xts[ti // 2]
                    tloc = (ti % 2) * TT
                    gps = [pps.tile([128, 512], dt.float32, tag="ps",
                                    name=f"g{t}_{j}") for j in range(3)]
                    ups = [pps.tile([128, 512], dt.float32, tag="ps",
                                    name=f"u{t}_{j}") for j in range(3)]
                    xtr = xt.rearrange("p (hb t) -> p hb t", hb=HB)
                    for k in range(KP):
                        lhs = xtr[:, 2 * k:2 * k + 2, tloc:tloc + TT]
                        st, sp = (k == 0), (k == KP - 1)
                        for wi, ph in ((0, gps), (1, ups)):
                            pair = (wq8[:, (wi * HB + 2 * k) * IC:
                                        (wi * HB + 2 * k + 2) * IC]
                                    .rearrange("p (j i) -> p j i", j=2))
                            for j, (i0, iw) in enumerate(THIRDS):
                                nc.tensor.matmul(
                                    ph[j][:, :iw], lhs,
                                    pair[:, :, i0:i0 + iw],
                                    start=st, stop=sp,
                                    perf_mode=mybir.MatmulPerfMode.DoubleRow)
                    pt = pp_pool.tile([128, IC], dt.float32, tag="pp", name=f"pt{t}")
                    for j, (i0, iw) in enumerate(THIRDS):
                        rt = pr.tile([128, 512], dt.float32, tag="rt",
                                     name=f"rt{t}_{j}")
                        nc.scalar.activation(rt[:, :iw], gps[j][:, :iw], Act.Relu)
                        nc.vector.scalar_tensor_tensor(
                            out=pt[:, i0:i0 + iw], in0=rt[:, :iw], scalar=1.0,
                            in1=ups[j][:, :iw], op0=Alu.mult, op1=Alu.mult)
                        nc.vector.tensor_tensor(
                            out=pt[:, i0:i0 + iw], in0=pt[:, i0:i0 + iw],
                            in1=rt[:, :iw], op=Alu.mult)
                    nc.vector.tensor_reduce(out=gam_p[:, t:t + 1], in_=pt[:],
                                            axis=mybir.AxisListType.X, op=Alu.max,
                                            apply_absolute_value=True)
                    pts[t] = pt

            def gamma_ar(b):
                sl0, sl1 = b * NTS, (b + 1) * NTS
                nc.sync.dma_start(gpart_d[b][:], gam_p[:, sl0:sl1])
                nc.gpsimd.collective_compute("AllReduce", Alu.max, replica_groups=RG,
                                             ins=[gpart_d[b].opt()],
                                             outs=[gall_d[b].opt()])
                nc.gpsimd.dma_start(gam[:, sl0:sl1], gall_d[b][:])

            def gamma_scales(b):
                sl0, sl1 = b * NTS, (b + 1) * NTS
                t1 = pr.tile([128, NTS], dt.float32, tag="gsc", name=f"gs{b}a")
                nc.vector.tensor_tensor(out=t1[:], in0=beta_all[:, sl0:sl1],
                                        in1=beta_all[:, sl0:sl1], op=Alu.mult)
                nc.vector.tensor_tensor(out=t1[:], in0=t1[:],
                                        in1=beta_all[:, sl0:sl1], op=Alu.mult)
                nc.vector.tensor_scalar(out=t1[:], in0=t1[:], scalar1=cb[:],
                                        scalar2=None, op0=Alu.mult)  # Ct
                cgs = pr.tile([128, NTS], dt.float32, tag="gsc2", name=f"gs{b}b")
                nc.vector.tensor_tensor(out=cgs[:], in0=t1[:], in1=gam[:, sl0:sl1],
                                        op=Alu.mult)  # C*gam
                rn = pr.tile([128, NTS], dt.float32, tag="gsc3", name=f"gs{b}c")
                nc.vector.tensor_scalar(out=rn[:], in0=cgs[:], scalar1=EPS,
                                        scalar2=None, op0=Alu.add)
                nc.vector.reciprocal(rn[:], rn[:])
                nc.vector.tensor_scalar(out=t1[:], in0=t1[:], scalar1=127.0,
                                        scalar2=None, op0=Alu.mult)
                nc.vector.tensor_tensor(out=s2[:, sl0:sl1], in0=t1[:], in1=rn[:],
                                        op=Alu.mult)
                nc.vector.tensor_scalar(out=cgs[:], in0=cgs[:], scalar1=cdd[:],
                                        scalar2=None, op0=Alu.mult)
                nc.vector.tensor_scalar(out=Dt[:, sl0:sl1], in0=cgs[:],
                                        scalar1=1.0 / 127.0, scalar2=None,
                                        op0=Alu.mult)

            def quant_super(b):
                iqT = piqT.tile([128, NSLAB * TSUP], dt.bfloat16, tag="iqT",
                                name=f"iqT_{b}")
                for ti in range(NTS):
                    t = b * NTS + ti
                    pt = pts.pop(t)
                    nc.scalar.activation(pt[:], pt[:], Act.Copy, bias=MAGIC,
                                         scale=s2[:, t:t + 1])
                    nc.vector.tensor_scalar(out=pt[:], in0=pt[:],
                                            scalar1=MAGIC + 127.0,
                                            scalar2=MAGIC - 128.0,
                                            op0=Alu.min, op1=Alu.max)
                    qt = ppiq.tile([128, ICP], dt.bfloat16, tag="piq", name=f"qt{t}")
                    nc.vector.tensor_scalar(out=qt[:, 0:IC], in0=pt[:],
                                            scalar1=-MAGIC, scalar2=None, op0=Alu.add)
                    nc.vector.memset(qt[:, IC:ICP], 0.0)
                    nc.sync.dma_start(iq_d[b][ti * TT:(ti + 1) * TT, :], qt[:])
                for sb in range(NSLAB):
                    nc.scalar.dma_start_transpose(iqT[:, sb * TSUP:(sb + 1) * TSUP],
                                                  iq_d[b][:, sb * 128:(sb + 1) * 128])
                return iqT

            # ===================== mm2 (super pairs) ========================
            def mm2_pair(m, iqTs):
                for hh in range(8):
                    ops = [pps.tile([128, 512], dt.float32, tag="ps",
                                    name=f"o{m}_{hh}_{q}") for q in range(8)]
                    for k in range(NSLAB):
                        slab = pwd.tile([128, 512], dt.bfloat16, tag="pwd",
                                        name=f"wds{m}_{hh}_{k}")
                        nc.sync.dma_start(slab[:],
                                          wdT_d[k * 128:(k + 1) * 128,
                                                hh * 512:(hh + 1) * 512])
                        st, sp = (k == 0), (k == NSLAB - 1)
                        for q in range(8):
                            lhsT = iqTs[q // 4][:, k * TSUP + (q % 4) * TT:
                                                k * TSUP + (q % 4 + 1) * TT]
                            nc.tensor.matmul(ops[q][:], lhsT, slab[:],
                                             start=st, stop=sp)
                    for q in range(8):
                        bq = 2 * m + q // 4
                        t = bq * NTS + (q % 4)
                        ev = pev.tile([128, 512], dt.bfloat16, tag="pev",
                                      name=f"ev{m}_{hh}_{q}")
                        nc.scalar.activation(ev[:], ops[q][:], Act.Copy,
                                             scale=Dt[:, t:t + 1])
                        nc.sync.dma_start(
                            part_d[bq][(q % 4) * TT:(q % 4 + 1) * TT,
                                       hh * 512:(hh + 1) * 512], ev[:])

            def rs_super(b):
                nc.gpsimd.collective_compute("ReduceScatter", Alu.add,
                                             replica_groups=RG,
                                             ins=[part_d[b].opt()],
                                             outs=[rs_d[b].opt()])
                nc.gpsimd.dma_start(out_ext.ap()[b], rs_d[b][:])

            # ===================== emission =================================
            xts0 = prep_x(0)
            w_scan(wg_in, IC, 0)
            for ti in range(NTS):
                prep_x_quant(1, ti)
            w_scan(wu_in, IC, 1)
            w_quant(wg_in, IC, 0, wgt_d)
            w_transpose_gu(0, wgt_d)
            w_quant(wu_in, IC, 1, wut_d)
            w_transpose_gu(1, wut_d)
            nc.vector.tensor_tensor(out=cgg[:], in0=wmv[:, 0:1], in1=sbc[:, 0:1],
                                    op=Alu.mult)
            nc.vector.tensor_tensor(out=cuu[:], in0=wmv[:, 1:2], in1=sbc[:, 1:2],
                                    op=Alu.mult)
            nc.vector.tensor_tensor(out=cb[:], in0=cgg[:], in1=cgg[:], op=Alu.mult)
            nc.vector.tensor_tensor(out=cb[:], in0=cb[:], in1=cuu[:], op=Alu.mult)

            mm1_super(0, xts0)
            xts_cur = [prep_x_transpose(1, 0), prep_x_transpose(1, 1)]
            gamma_ar(0)
            w_scan(wd_in, H, 2)
            nc.vector.tensor_tensor(out=cdd[:], in0=wmv[:, 2:3], in1=sbc[:, 2:3],
                                    op=Alu.mult)
            w_quant(wd_in, H, 2, wdt_d)
            w_transpose_d()
            gamma_scales(0)
            iqts = {0: quant_super(0)}

            for b in range(1, NSUP):
                mm1_super(b, xts_cur)
                gamma_ar(b)
                if b + 1 < NSUP:
                    xts_cur = prep_x(b + 1)
                if b % 2 == 0:
                    m = b // 2 - 1
                    mm2_pair(m, (iqts.pop(2 * m), iqts.pop(2 * m + 1)))
                    rs_super(2 * m)
                    rs_super(2 * m + 1)
                gamma_scales(b)
                iqts[b] = quant_super(b)
            mm2_pair(3, (iqts.pop(6), iqts.pop(7)))
            rs_super(6)
            rs_super(7)

            if debug:
                nc.gpsimd.dma_start(dbg["dbg_beta"].ap(), beta_all[:])
                nc.gpsimd.dma_start(dbg["dbg_gam"].ap(), gam[:])
                nc.gpsimd.dma_start(dbg["dbg_s2"].ap(), s2[:])
                nc.gpsimd.dma_start(dbg["dbg_Dt"].ap(), Dt[:])
                nc.gpsimd.dma_start(dbg["dbg_wm"].ap(), wmv[0:1, :])
                nc.gpsimd.dma_start(dbg["dbg_xq"].ap(), xq_d[0][:])
                nc.gpsimd.dma_start(dbg["dbg_iq"].ap(), iq_d[0][:])
                nc.gpsimd.dma_start(dbg["dbg_part"].ap(), part_d[0][:])

    nc.compile()
    return nc


def _get_compiled(debug=False):
    key = ("nc", debug)
    if key not in _cache:
        _cache[key] = _build(debug)
    return _cache[key]


def make_in_maps(x, w_gate, w_up, w_down, s_gate, s_up, s_down):
    xf = np.ascontiguousarray(np.asarray(x).reshape(T, H).astype(np.float32,
                                                                 copy=False))
    scales = np.array([[float(np.asarray(s_gate).reshape(-1)[0]),
                        float(np.asarray(s_up).reshape(-1)[0]),
                        float(np.asarray(s_down).reshape(-1)[0])]],
                      dtype=np.float32)
    in_maps = []
    for c in range(N_CORES):
        i0 = c * IC
        in_maps.append({
            "x": xf,
            "wg": np.ascontiguousarray(w_gate[i0:i0 + IC, :], dtype=np.float32),
            "wu": np.ascontiguousarray(w_up[i0:i0 + IC, :], dtype=np.float32),
            "wd": np.ascontiguousarray(w_down[:, i0:i0 + IC], dtype=np.float32),
            "scales": scales,
        })
    return in_maps


def assemble_out(results):
    out = np.empty((T, H), dtype=np.float32)
    tpc = TSUP // N_CORES
    for c in range(N_CORES):
        o = np.asarray(results[c]["out_rs"]).astype(np.float32)
        for b in range(NSUP):
            out[b * TSUP + c * tpc: b * TSUP + (c + 1) * tpc] = o[b]
    return out.reshape(B, S, H)


def kernel(x, w_gate, w_up, w_down, s_gate, s_up, s_down):
    from concourse.bass_utils import run_bass_kernel_spmd

    nc = _get_compiled()
    in_maps = make_in_maps(x, w_gate, w_up, w_down, s_gate, s_up, s_down)
    res = run_bass_kernel_spmd(nc, in_maps, core_ids=list(range(N_CORES)))
    return assemble_out(res.results)
